# revision 19
# baseline (speedup 1.0000x reference)
"""Trainium2 Bass kernel for MoEPred: softmax-gated mixture of 32 tiny experts.

  xi[b] = sum_e softmax_e(x@Wg.T) * (W2[e] . gelu(x @ W1[e] + b1[e]) + b2[e])

Sharding: pure data parallel over batch across 8 NeuronCores; weights
replicated. x is pre-laid-out on the host so each 512-row macro-tile is one
contiguous chunk of a 4MB group DMA landing as xT chunks [feat 128, rows 512]
(the contraction dim on SBUF partitions).

Orientation (the key choice): MM1 runs TRANSPOSED — the stationary operand is
the x chunk [128 feat, 128 rows] and the moving operand is W1 [128 feat,
512 eh], so h lands as hp[rows, eh]. Benefits vs the weight-stationary form:
  * the gating matmul becomes a free "rider": it reuses the exact stationary
    x chunk with a tiny moving operand Wg [128, 32] -> N=32 stream (~60 cycle
    floor) instead of 4 full 512-cycle streams per macro;
  * MM2 (block-diagonal, 16-wide per expert) leaves the PE entirely: with h
    as [rows, eh] it is an elementwise multiply by a replicated W2 row plus a
    segmented (groups of 16) free-dim reduce on the DVE;
  * the expert-weighted softmax combine is all free-dim reduces on DVE.
PE work per 512-row macro drops from 24.5 to 16 full streams + 16 tiny
riders (~3.6us vs ~5.3us).

Schedule: per macro, two [128,1024] PSUM halves; each half = 2 row-chunks x
4 k-chunk accumulation (big MM + gating rider share each LDWEIGHTS). DVE adds
b1 (bias varies along eh = free dim, so ACT's per-partition bias can't do it),
ACT gelus one [128,1024] tile per half. Gating logits for 4 macros pack into
one PSUM bank ([128, 16*32]); exp runs for two such banks back-to-back every
8 macros so the gelu<->exp ACT table-set switch (~1.3us) amortizes to
2 loads / 8 macros. The combine (o2+b2)*eg, num/den segmented reduces,
reciprocal and xi multiply run on DVE per 4-macro group; xi accumulates in a
[128, n_macro*4] SBUF tile DMA'd once at the end (host reorders).
"""

import os
import sys
from contextlib import ExitStack

import numpy as np

for _p in ("/opt/trn_rl_repo",):
    if _p not in sys.path:
        sys.path.insert(0, _p)

import jax
from jax.experimental.shard_map import shard_map
from jax.sharding import Mesh, NamedSharding, PartitionSpec

import concourse.bacc as bacc
import concourse.bass2jax as b2j
import concourse.tile as tile
from concourse import mybir

N_CORES = 8
BATCH = 262144
D_IN = 512
N_EXPERTS = 32
HID = 16
EH = N_EXPERTS * HID  # 512
B_LOC = BATCH // N_CORES  # 32768
R = 512  # rows per macro-tile
KC = D_IN // 128  # 4 feature chunks
RC = R // 128  # 4 row chunks per macro

F32 = mybir.dt.float32
BF16 = mybir.dt.bfloat16
AF = mybir.ActivationFunctionType
ALU = mybir.AluOpType
AX = mybir.AxisListType

_NC_CACHE = {}
_RUNNER_CACHE = {}


def build_nc(b_loc=B_LOC, loop_n=1, dma_group=4, xq_bufs=2, riders=True):
    """loop_n > 1 wraps the macro loop in a hardware For_i that redoes the
    identical work loop_n times (benchmark amplification above the ~80-100ms
    axon dispatch floor). riders=False drops the gating matmuls (wrong
    results; used only to time the MM1 skeleton)."""
    assert b_loc % R == 0
    n_macro = b_loc // R  # 64
    assert n_macro % 8 == 0
    G = min(dma_group, n_macro)
    assert n_macro % G == 0
    n_group = n_macro // G

    nc = bacc.Bacc("TRN2", target_bir_lowering=False, debug=False,
                   num_devices=N_CORES)

    xTm = nc.dram_tensor("xTm", [n_group * 128, G * KC * R], BF16,
                         kind="ExternalInput")
    w1t = nc.dram_tensor("w1t", [D_IN, EH], BF16, kind="ExternalInput")
    wgt = nc.dram_tensor("wgt", [D_IN, N_EXPERTS], BF16, kind="ExternalInput")
    b1r2 = nc.dram_tensor("b1r2", [128, 2 * EH], F32, kind="ExternalInput")
    w2r4 = nc.dram_tensor("w2r4", [128, 4 * EH], BF16, kind="ExternalInput")
    b2r16 = nc.dram_tensor("b2r16", [128, 512], F32, kind="ExternalInput")
    outT = nc.dram_tensor("outT", [128, n_macro * RC], F32,
                          kind="ExternalOutput")

    with tile.TileContext(nc) as tc, ExitStack() as ctx:
        const = ctx.enter_context(tc.tile_pool(name="const", bufs=1))
        xpool = ctx.enter_context(tc.tile_pool(name="xp", bufs=xq_bufs))
        tmppool = ctx.enter_context(tc.tile_pool(name="tmpp", bufs=4))
        hapool = ctx.enter_context(tc.tile_pool(name="hap", bufs=3))
        st2pool = ctx.enter_context(tc.tile_pool(name="st2p", bufs=3))
        redpool = ctx.enter_context(tc.tile_pool(name="redp", bufs=2))
        o2pool = ctx.enter_context(tc.tile_pool(name="o2p", bufs=4))
        egpool = ctx.enter_context(tc.tile_pool(name="egp", bufs=2))
        cbpool = ctx.enter_context(tc.tile_pool(name="cbp", bufs=2))
        smpool = ctx.enter_context(tc.tile_pool(name="smp", bufs=4))
        xoutpool = ctx.enter_context(tc.tile_pool(name="xout", bufs=1))
        ps_h = ctx.enter_context(tc.tile_pool(name="ps_h", bufs=2, space="PSUM"))
        ps_g = ctx.enter_context(tc.tile_pool(name="ps_g", bufs=4, space="PSUM"))

        # --- replicated constants, loaded once ---
        w1_sb = const.tile([128, KC * EH], BF16, name="w1_sb")
        wg_sb = const.tile([128, KC * N_EXPERTS], BF16, name="wg_sb")
        b1rep = const.tile([128, 2 * EH], F32, name="b1rep")
        w2rep = const.tile([128, 4 * EH], BF16, name="w2rep")
        b2rep = const.tile([128, 512], F32, name="b2rep")
        # consts on the ACT DMA queue so they don't serialize ahead of the
        # first x-group DMA on the sync queue (ACT is idle at kernel start)
        for k in range(KC):
            nc.scalar.dma_start(w1_sb[:, k * EH:(k + 1) * EH],
                                w1t[k * 128:(k + 1) * 128, :])
            nc.scalar.dma_start(wg_sb[:, k * 32:(k + 1) * 32],
                                wgt[k * 128:(k + 1) * 128, :])
        nc.scalar.dma_start(b1rep[:], b1r2[:, :])
        nc.scalar.dma_start(w2rep[:], w2r4[:, :])
        nc.scalar.dma_start(b2rep[:], b2r16[:, :])

        if loop_n > 1:
            ctx.enter_context(tc.For_i(0, loop_n, 1))

        xi_all = xoutpool.tile([128, n_macro * RC], F32, name="xi_all")

        # ---- x group-DMA prefetch ----
        xq_tiles = {}

        def issue_group_dma(grp):
            if grp >= n_group:
                return
            xq_t = xpool.tile([128, G * KC * R], BF16, tag="xq", name="xq")
            nc.sync.dma_start(xq_t[:], xTm[grp * 128:(grp + 1) * 128, :])
            xq_tiles[grp] = xq_t

        issue_group_dma(0)

        pend = []  # up to two 4-macro groups awaiting exp+combine

        def emit_exp(gp):
            egm = egpool.tile([128, 512], BF16, tag="egm", name="egm")
            nc.scalar.activation(egm[:], gp[:], AF.Exp)
            return egm

        def combine_dve(egm, o2m, gidx):
            t1 = cbpool.tile([128, 512], BF16, tag="t1", name="t1")
            t2 = cbpool.tile([128, 512], BF16, tag="t2", name="t2")
            nc.vector.tensor_add(t1[:], o2m[:], b2rep[:])
            nc.vector.tensor_mul(t2[:], t1[:], egm[:])
            num = smpool.tile([128, 16], F32, tag="num", name="num")
            den = smpool.tile([128, 16], F32, tag="den", name="den")
            nc.vector.tensor_reduce(
                num[:], t2[:].rearrange("p (s e) -> p s e", e=32),
                axis=AX.X, op=ALU.add)
            nc.vector.tensor_reduce(
                den[:], egm[:].rearrange("p (s e) -> p s e", e=32),
                axis=AX.X, op=ALU.add)
            nc.vector.reciprocal(den[:], den[:])
            nc.vector.tensor_mul(xi_all[:, gidx * 16:(gidx + 1) * 16],
                                 num[:], den[:])

        def emit_reduce(st2, o2m, jj4):
            """Segmented sum over HID=16 as 4 pairwise log-rounds: packed bf16
            APs qualify for the DVE 2x_1p mode (flat InstTensorReduce has no
            fast mode)."""
            r1 = redpool.tile([128, 1024], BF16, tag="r1", name="r1")
            r2 = redpool.tile([128, 512], BF16, tag="r2", name="r2")
            r3 = redpool.tile([128, 256], BF16, tag="r3", name="r3")
            s3 = st2[:].rearrange("p (s h) -> p s h", h=HID)
            v1 = r1[:].rearrange("p (s h) -> p s h", h=8)
            nc.vector.tensor_add(v1, s3[:, :, 0:8], s3[:, :, 8:16])
            v2 = r2[:].rearrange("p (s h) -> p s h", h=4)
            nc.vector.tensor_add(v2, v1[:, :, 0:4], v1[:, :, 4:8])
            v3 = r3[:].rearrange("p (s h) -> p s h", h=2)
            nc.vector.tensor_add(v3, v2[:, :, 0:2], v2[:, :, 2:4])
            nc.vector.tensor_add(
                o2m[:, jj4 * 128:(jj4 + 1) * 128].rearrange(
                    "p (s h) -> p s h", h=1),
                v3[:, :, 0:1], v3[:, :, 1:2])

        # Software-pipelined emission: engine queues are strict FIFO, so every
        # op must be (nearly) ready when the engine reaches it. The log-reduce
        # of macro j waits on GpSimd's mult(j); emitting it during macro j+1
        # keeps it from head-blocking the PSUM-freeing bias-evict of j+1.
        # Combines (2 per 8 macros) are likewise lagged and split across
        # macros to avoid a DVE burst in front of the next bias-evict.
        gp = o2m = None
        pend_red = []   # (st2, o2m, jj4) awaiting log-reduce, lag 1 macro
        pend_exp = []   # (gp, o2m, gidx) groups awaiting exp
        pend_cmb = []   # (egm, o2m, gidx) awaiting DVE combine, 1/macro
        for j in range(n_macro):
            if j % G == 0:
                grp = j // G
                issue_group_dma(grp + 1)
                xq = xq_tiles.pop(grp)
            xj = xq[:, (j % G) * KC * R:(j % G + 1) * KC * R]

            # both groups of a supergroup exp'd adjacently on ACT -> only
            # 2 gelu<->exp table switches (~1.3us each) per 8 macros.
            # Emitted BEFORE the next gp/o2m allocations so the slot-frees
            # are already in the schedule when the allocator waits on them.
            if len(pend_exp) == 2:
                for gp_p, o2m_p, gidx_p in pend_exp:
                    pend_cmb.append((j, emit_exp(gp_p), o2m_p, gidx_p))
                pend_exp.clear()

            jj4 = j % 4
            if jj4 == 0:
                gp = ps_g.tile([128, 512], F32, tag="gp", name="gp")
                o2m = o2pool.tile([128, 512], F32, tag="o2m", name="o2m")

            ha = hapool.tile([128, 4 * EH], BF16, tag="ha", name="ha")
            for half in range(2):
                hp = ps_h.tile([128, 2 * EH], F32, tag="hp", name="hp")
                for ci in range(2):
                    c = 2 * half + ci
                    for k in range(KC):
                        lhsT = xj[:, k * R + c * 128:k * R + (c + 1) * 128]
                        nc.tensor.matmul(
                            hp[:, ci * EH:(ci + 1) * EH],
                            lhsT=lhsT,
                            rhs=w1_sb[:, k * EH:(k + 1) * EH],
                            start=(k == 0), stop=(k == KC - 1),
                            skip_group_check=True)
                        if riders:
                            nc.tensor.matmul(
                                gp[:, (jj4 * 4 + c) * 32:(jj4 * 4 + c + 1) * 32],
                                lhsT=lhsT,
                                rhs=wg_sb[:, k * 32:(k + 1) * 32],
                                start=(k == 0), stop=(k == KC - 1),
                                skip_group_check=True)
                tmp = tmppool.tile([128, 2 * EH], BF16, tag="tmp", name="tmp")
                nc.vector.tensor_add(tmp[:], hp[:], b1rep[:])
                nc.scalar.activation(ha[:, half * 2 * EH:(half + 1) * 2 * EH],
                                     tmp[:], AF.Gelu)

            # w2-multiply on GpSimd: its SBUF port is DVE's *second* (perf-mode)
            # port, so this runs concurrently with DVE's dedicated-port passes.
            st2 = st2pool.tile([128, 4 * EH], BF16, tag="st2", name="st2")
            nc.gpsimd.tensor_mul(st2[:], ha[:], w2rep[:])

            if pend_red:
                emit_reduce(*pend_red.pop())
            pend_red.append((st2, o2m, jj4))
            # combine lags >=1 macro behind its exp so the ACT table-load
            # latency never head-blocks the DVE queue
            if pend_cmb and pend_cmb[0][0] < j:
                _, egm_p, o2m_p, gidx_p = pend_cmb.pop(0)
                combine_dve(egm_p, o2m_p, gidx_p)
            if jj4 == 3:
                pend_exp.append((gp, o2m, j // 4))

        # drain
        while pend_red:
            emit_reduce(*pend_red.pop())
        for gp_p, o2m_p, gidx_p in pend_exp:
            pend_cmb.append((0, emit_exp(gp_p), o2m_p, gidx_p))
        pend_exp.clear()
        while pend_cmb:
            _, egm_p, o2m_p, gidx_p = pend_cmb.pop(0)
            combine_dve(egm_p, o2m_p, gidx_p)

        nc.sync.dma_start(outT[:, :], xi_all[:])

    nc.compile()
    return nc


def prep_weights(Wg, W1, b1, W2, b2, np_dt=np.float32):
    w1t = np.ascontiguousarray(
        np.asarray(W1, dtype=np.float32).transpose(1, 0, 2).reshape(D_IN, EH)
    ).astype(np_dt)
    wgt = np.ascontiguousarray(np.asarray(Wg, dtype=np.float32).T).astype(np_dt)
    b1flat = np.asarray(b1, dtype=np.float32).reshape(EH)
    b1r2 = np.ascontiguousarray(
        np.broadcast_to(np.tile(b1flat, 2)[None, :], (128, 2 * EH))).astype(
            np.float32)
    w2flat = np.asarray(W2, dtype=np.float32).reshape(EH)
    w2r4 = np.ascontiguousarray(
        np.broadcast_to(np.tile(w2flat, 4)[None, :], (128, 4 * EH))).astype(np_dt)
    b2flat = np.asarray(b2, dtype=np.float32).reshape(N_EXPERTS)
    b2r16 = np.ascontiguousarray(
        np.broadcast_to(np.tile(b2flat, 16)[None, :], (128, 512))).astype(
            np.float32)
    return {"w1t": w1t, "wgt": wgt, "b1r2": b1r2, "w2r4": w2r4, "b2r16": b2r16}


def layout_x(xc, np_dt=np.float32, dma_group=4):
    """Core shard [B_LOC, D_IN] -> per-group contiguous transposed layout
    [n_group*128, G*KC*R]: xTm[g*128+p, ((i*KC)+k)*R+c] = xc[(g*G+i)*R+c, k*128+p]."""
    n_macro = xc.shape[0] // R
    G = min(dma_group, n_macro)
    n_group = n_macro // G
    return np.ascontiguousarray(
        xc.reshape(n_group, G, R, KC, 128).transpose(0, 4, 1, 3, 2).reshape(
            n_group * 128, G * KC * R)).astype(np_dt)


class Runner:
    """Reusable SPMD executor: the multi-core path of
    concourse.bass2jax.run_bass_via_pjrt, factored so the jitted callable and
    device-resident inputs can be reused across calls (for benchmarking)."""

    def __init__(self, nc, n_cores=N_CORES):
        b2j.install_neuronx_cc_hook()
        self.nc = nc
        self.n_cores = n_cores
        partition_name = (
            nc.partition_id_tensor.name if nc.partition_id_tensor else None
        )
        in_names, out_names, out_avals, zero_outs = [], [], [], []
        for alloc in nc.m.functions[0].allocations:
            if not isinstance(alloc, mybir.MemoryLocationSet):
                continue
            assert alloc.memorylocations
            name = alloc.memorylocations[0].name
            if alloc.kind == "ExternalInput":
                if name != partition_name:
                    in_names.append(name)
            elif alloc.kind == "ExternalOutput":
                out_names.append(name)
                shape = tuple(alloc.tensor_shape)
                dtype = mybir.dt.np(alloc.dtype)
                out_avals.append(jax.core.ShapedArray(shape, dtype))
                zero_outs.append(np.zeros(shape, dtype))
        self.in_names = list(in_names)
        self.out_names = out_names
        self.zero_outs = zero_outs
        n_params = len(in_names)
        n_outs = len(out_names)
        bind_names = in_names + out_names
        if partition_name is not None:
            bind_names.append(partition_name)

        def _body(*args):
            operands = list(args)
            if partition_name is not None:
                operands.append(b2j.partition_id_tensor())
            outs = b2j._bass_exec_p.bind(
                *operands,
                out_avals=tuple(out_avals),
                in_names=tuple(bind_names),
                out_names=tuple(out_names),
                lowering_input_output_aliases=(),
                sim_require_finite=True,
                sim_require_nnan=True,
                nc=nc,
            )
            return tuple(outs)

        devices = jax.devices()[:n_cores]
        assert len(devices) == n_cores
        self.mesh = Mesh(np.asarray(devices), ("core",))
        in_specs = (PartitionSpec("core"),) * (n_params + n_outs)
        out_specs = (PartitionSpec("core"),) * n_outs
        self.fn = jax.jit(
            shard_map(_body, mesh=self.mesh, in_specs=in_specs,
                      out_specs=out_specs, check_rep=False),
            donate_argnums=tuple(range(n_params, n_params + n_outs)),
            keep_unused=True,
        )
        self.sharding = NamedSharding(self.mesh, PartitionSpec("core"))

    def put_inputs(self, in_maps):
        assert len(in_maps) == self.n_cores
        concat = [
            np.concatenate([np.asarray(m[name]) for m in in_maps], axis=0)
            for name in self.in_names
        ]
        return [jax.device_put(a, self.sharding) for a in concat]

    def fresh_outs(self):
        return [
            jax.device_put(
                np.zeros((self.n_cores * z.shape[0], *z.shape[1:]), z.dtype),
                self.sharding,
            )
            for z in self.zero_outs
        ]

    def run(self, dev_inputs, dev_outs=None):
        if dev_outs is None:
            dev_outs = self.fresh_outs()
        return self.fn(*dev_inputs, *dev_outs)


def get_runner(b_loc=B_LOC):
    if b_loc not in _RUNNER_CACHE:
        if b_loc not in _NC_CACHE:
            _NC_CACHE[b_loc] = build_nc(b_loc)
        _RUNNER_CACHE[b_loc] = Runner(_NC_CACHE[b_loc])
    return _RUNNER_CACHE[b_loc]


def make_in_maps(x, Wg, W1, b1, W2, b2, np_dt=None, dma_group=4):
    if np_dt is None:
        import ml_dtypes
        np_dt = ml_dtypes.bfloat16
    x = np.asarray(x, dtype=np.float32)
    consts = prep_weights(Wg, W1, b1, W2, b2, np_dt)
    xs = x.reshape(N_CORES, B_LOC, D_IN)
    in_maps = []
    for i in range(N_CORES):
        m = dict(consts)
        m["xTm"] = layout_x(xs[i], np_dt, dma_group)
        in_maps.append(m)
    return in_maps


def kernel(x, Wg, W1, b1, W2, b2):
    os.environ["BASS_NEVER_TRACE"] = "1"
    in_maps = make_in_maps(x, Wg, W1, b1, W2, b2)
    runner = get_runner(B_LOC)
    dev_in = runner.put_inputs(in_maps)
    outs = runner.run(dev_in)
    out_t = np.asarray(outs[0])  # [N_CORES*128, n_macro*RC]
    n_mr = out_t.shape[1]
    per_core = out_t.reshape(N_CORES, 128, n_mr).transpose(0, 2, 1)
    return np.ascontiguousarray(per_core.reshape(BATCH, 1))


if __name__ == "__main__":
    rng = np.random.default_rng(0)
    inputs = {
        "x": rng.standard_normal((BATCH, D_IN), dtype=np.float32),
        "Wg": (rng.standard_normal((N_EXPERTS, D_IN)) * 0.02).astype(np.float32),
        "W1": (rng.standard_normal((N_EXPERTS, D_IN, HID)) * 0.02).astype(np.float32),
        "b1": (rng.standard_normal((N_EXPERTS, HID)) * 0.02).astype(np.float32),
        "W2": (rng.standard_normal((N_EXPERTS, HID)) * 0.02).astype(np.float32),
        "b2": (rng.standard_normal((N_EXPERTS,)) * 0.02).astype(np.float32),
    }
    out = kernel(**inputs)
    print(out.shape, out.dtype, out[:4, 0])
